# revision 1
# baseline (speedup 1.0000x reference)
"""CapsNet DigitCaps routing kernel for 8 TRN2 NeuronCores.

Strategy: shard the 1152 primary capsules across the 8 cores (144 each),
keep the full batch (256) on every core. Each core builds its slice of
u_hat = einsum('dpij,bpj->bdpi') once on the TensorEngine (block-diagonal
weight trick: 3 primary capsules per matmul, K=32 with zero padding,
4-way tile_position row-tiling, N=480 = one PSUM bank) and keeps it
resident in SBUF as bf16 (11.8MB). The 3 dynamic-routing iterations then
run entirely from SBUF: the two batched contractions per iteration
(s = sum_p c*u_hat and the agreement logits g = sum_i u_hat*v) are DVE
broadcast-multiplies at the 2x bf16 perf mode followed by halving
add-trees; softmax over the 10 digit capsules runs on ACT(exp)+DVE with
d kept innermost. Iteration 0's s comes straight from one K=9216 matmul
chain (uniform c). sqrt for the squash scale is exp(0.5*ln(n2)) so the
whole kernel uses a single ACT table set (natural_log_exp).

The reference's _squash uses a GLOBAL Frobenius norm over the whole
[B,D,1,16] s tensor, which couples all batch elements and hence all
shards: each iteration does one small (164KB) AllReduce of the per-core
partial s, after which every core computes the identical squash scale
alpha = n2 / ((n2+1) * (sqrt(n2)+eps)) and proceeds identically.

Layouts (per core, SBUF partition dim first):
  u_hat  [128, 2*48*480] bf16   row=b%128, col = bt*23040 + g*480 + ph*160 + i*10 + d
                                 (bt=b//128, p_local = 3*g+ph)
  L,c    [128, 2*1440]          col = bt*1440 + p*10 + d
  s,sv   [128, 2*160]  f32      col = bt*160 + i*10 + d
"""

import os
import sys

for _p in ("/opt/trn_rl_repo",):
    if _p not in sys.path and os.path.isdir(_p):
        sys.path.insert(0, _p)

import numpy as np
import ml_dtypes

import concourse.bass as bass
import concourse.bacc as bacc
import concourse.mybir as mybir
import concourse.tile as tile
from concourse.bass_utils import run_bass_kernel_spmd

F32 = mybir.dt.float32
BF16 = mybir.dt.bfloat16
MULT = mybir.AluOpType.mult
ADD = mybir.AluOpType.add
AF = mybir.ActivationFunctionType

B, D, P, I, J = 256, 10, 1152, 16, 8
CORES = 8
PL = P // CORES          # 144 local primary capsules
G = PL // 3              # 48 groups of 3 (block-diag build, N=480 = one PSUM bank)
GW = 3 * I * D           # 480 cols per group
NQ = G // 4              # 12 q-tiles of 4 groups stacked on 128 partitions
NKT = (PL * J) // 128    # 9 k-tiles of 128 (p,j)-rows
EPS = 1e-7
NROUT = 3

# Set BASSCAPS_TREE_F32=1 to run the p/i reduction trees in fp32.
TREE_F32 = os.environ.get("BASSCAPS_TREE_F32", "0") == "1"
# Debug bisection: 0=build+s0, 1=+AllReduce, 2=+alpha/sv, 3=+g/L/softmax, 4=full
STAGE = int(os.environ.get("BASSCAPS_STAGE", "4"))
# Replace the AllReduce with a local DRAM copy (for TimelineSim profiling).
NO_CC = os.environ.get("BASSCAPS_NO_CC", "0") == "1"


def build_program():
    nc = bacc.Bacc("TRN2", target_bir_lowering=False, debug=False,
                   num_devices=CORES)

    # Per-core DRAM inputs (host pre-arranged, bf16).
    xl_d = nc.dram_tensor("x_l", [128, NQ * 256], BF16, kind="ExternalInput")
    wbd_d = nc.dram_tensor("w_bd", [128, NQ * GW], BF16, kind="ExternalInput")
    xpj_d = nc.dram_tensor("x_pj", [128, NKT * 256], BF16, kind="ExternalInput")
    wfl_d = nc.dram_tensor("w_fl", [128, NKT * 160], BF16, kind="ExternalInput")
    v_d = nc.dram_tensor("v_out", [B, D, I], F32, kind="ExternalOutput")

    TDT = F32 if TREE_F32 else BF16

    with tile.TileContext(nc) as tc:
        with (
            tc.tile_pool(name="persist", bufs=1) as pp,
            tc.tile_pool(name="build", bufs=3) as bp,
            tc.tile_pool(name="psA", bufs=4, space=bass.MemorySpace.PSUM) as psA,
            tc.tile_pool(name="psB", bufs=2, space=bass.MemorySpace.PSUM) as psB,
            tc.tile_pool(name="dram", bufs=1, space=bass.MemorySpace.DRAM) as dp,
        ):
            uh = pp.tile([128, 2 * G * GW], BF16)
            Lb = pp.tile([128, 2 * 1440], F32)
            cpd = pp.tile([128, 2 * 1440], BF16)
            s_sb = pp.tile([128, 2 * 160], F32)
            sv = pp.tile([128, 2 * 160], F32)
            sv_bf = pp.tile([128, 2 * 160], BF16)
            Y = pp.tile([128, G * GW], TDT)        # per-b-tile scratch
            gf32 = pp.tile([128, 1440], F32)
            expL = pp.tile([128, 1440], F32)
            zbuf = pp.tile([128, 144], F32)
            zrec = pp.tile([128, 144], F32)
            acc = pp.tile([128, 1], F32)
            ones = pp.tile([128, 128], F32)
            ttr_junk = pp.tile([128, 320], F32)
            n2sb = pp.tile([128, 1], F32)
            t1 = pp.tile([128, 1], F32)
            r1 = pp.tile([128, 1], F32)
            lnv = pp.tile([128, 1], F32)
            rt = pp.tile([128, 1], F32)
            t2 = pp.tile([128, 1], F32)
            r2 = pp.tile([128, 1], F32)
            alpha_bc = pp.tile([128, 1], F32)

            bounce_in = dp.tile([B, 160], F32)
            bounce_out = dp.tile([B, 160], F32)

            nc.vector.memset(ones[:, :], 1.0)
            nc.gpsimd.memset(Lb[:, :], 0.0)

            # ---- load the small matmul operands for s0 ----
            xpj = bp.tile([128, NKT * 256], BF16, tag="xpj")
            wfl = bp.tile([128, NKT * 160], BF16, tag="wfl")
            nc.sync.dma_start(xpj[:, :], xpj_d.ap())
            nc.sync.dma_start(wfl[:, :], wfl_d.ap())

            # ---- s0 = 0.1 * sum_p u_hat  (direct from x, W) ----
            for bt in range(2):
                ps0 = psB.tile([128, 160], F32, tag="s0")
                for kt in range(NKT):
                    nc.tensor.matmul(
                        ps0[:, :],
                        xpj[:, kt * 256 + bt * 128: kt * 256 + bt * 128 + 128],
                        wfl[:, kt * 160:(kt + 1) * 160],
                        start=(kt == 0), stop=(kt == NKT - 1),
                    )
                nc.scalar.mul(s_sb[:, bt * 160:(bt + 1) * 160], ps0[:, :], 0.1)

            # ---- build u_hat: 4-way row-tiled block-diag matmuls ----
            # group g = 3 primary capsules; lhsT = x block [32, 128]
            # (rows = (ph, j), ph==3 padding), rhs = W block-diag [32, 480].
            for q in range(NQ):
                xlq = bp.tile([128, 256], BF16, tag="xl")
                wbq = bp.tile([128, GW], BF16, tag="wb")
                nc.sync.dma_start(xlq[:, :], xl_d.ap()[:, q * 256:(q + 1) * 256])
                nc.sync.dma_start(wbq[:, :], wbd_d.ap()[:, q * GW:(q + 1) * GW])
                for bt in range(2):
                    for gg in range(4):
                        g = q * 4 + gg
                        ps = psA.tile([128, GW], F32, tag="bld")
                        nc.tensor.matmul(
                            ps[:, :],
                            xlq[32 * gg:32 * (gg + 1),
                                bt * 128:(bt + 1) * 128],
                            wbq[32 * gg:32 * (gg + 1), :],
                            start=True, stop=True,
                            tile_position=(32 * gg, 0),
                        )
                        dst = uh[:, bt * (G * GW) + g * GW:
                                 bt * (G * GW) + (g + 1) * GW]
                        if g % 2 == 0:
                            nc.vector.tensor_copy(dst, ps[:, :])
                        else:
                            nc.scalar.copy(dst, ps[:, :])

            # block dims helper: (g, ph, i, d) iteration space
            def uh_ap(bt):
                return uh[:, bt * (G * GW):(bt + 1) * (G * GW)].rearrange(
                    "p (g ph i d) -> p g ph i d", g=G, ph=3, i=16, d=10)

            def y_ap():
                return Y[:, :].rearrange(
                    "p (g ph i d) -> p g ph i d", g=G, ph=3, i=16, d=10)

            for r in range(NROUT if STAGE >= 4 else 1):
                if r > 0:
                    # ---- s_partial = sum_p c * u_hat ----
                    for bt in range(2):
                        cb = cpd[:, bt * 1440:(bt + 1) * 1440].rearrange(
                            "p (pp d) -> p pp d", pp=144, d=10)
                        cb4 = cb.rearrange("p (g ph) d -> p g ph d", g=G, ph=3)
                        nc.vector.tensor_tensor(
                            y_ap(),
                            uh_ap(bt),
                            cb4[:, :, :, None, :].to_broadcast((128, G, 3, 16, 10)),
                            MULT,
                        )
                        # halving tree over p (144 blocks of 160)
                        yp = Y[:, :].rearrange("p (pp x) -> p pp x", pp=144, x=160)
                        for sz in (72, 36, 18, 9):
                            nc.vector.tensor_tensor(
                                yp[:, 0:sz, :], yp[:, 0:sz, :],
                                yp[:, sz:2 * sz, :], ADD)
                        nc.vector.tensor_tensor(
                            yp[:, 0:4, :], yp[:, 0:4, :], yp[:, 4:8, :], ADD)
                        nc.vector.tensor_tensor(
                            yp[:, 0:2, :], yp[:, 0:2, :], yp[:, 2:4, :], ADD)
                        nc.vector.tensor_tensor(
                            yp[:, 0:1, :], yp[:, 0:1, :], yp[:, 1:2, :], ADD)
                        nc.vector.tensor_tensor(
                            s_sb[:, bt * 160:(bt + 1) * 160],
                            yp[:, 0, :], yp[:, 8, :], ADD)

                # ---- AllReduce partial s over the 8 cores ----
                if STAGE < 1:
                    break
                nc.sync.dma_start(
                    bounce_in[:, :].rearrange("(t b) x -> b t x", t=2, b=128),
                    s_sb[:, :].rearrange("b (t x) -> b t x", t=2, x=160))
                if NO_CC:
                    nc.sync.dma_start(bounce_out[:, :], bounce_in[:, :])
                else:
                    nc.gpsimd.collective_compute(
                        "AllReduce", ADD,
                        ins=[bounce_in.opt()],
                        outs=[bounce_out.opt()],
                        replica_groups=[list(range(CORES))],
                    )
                nc.sync.dma_start(
                    s_sb[:, :].rearrange("b (t x) -> b t x", t=2, x=160),
                    bounce_out[:, :].rearrange("(t b) x -> b t x", t=2, b=128))

                if STAGE < 2:
                    break
                # ---- alpha = n2 / ((n2+1)(sqrt(n2)+eps)) , n2 = sum s^2 ----
                # ones[128,128] @ acc[128,1] lands n2 on all 128 partitions at
                # once, so the whole alpha chain runs [128,1]-wide and no
                # partition_broadcast is needed.
                nc.scalar.activation(ttr_junk[:, :], s_sb[:, :], AF.Square,
                                     accum_out=acc[:, :])
                psn = psB.tile([128, 1], F32, tag="n2")
                nc.tensor.matmul(psn[:, :], ones[:, :], acc[:, :],
                                 start=True, stop=True)
                nc.vector.tensor_copy(n2sb[:, :], psn[:, :])
                nc.vector.tensor_scalar_add(t1[:, :], n2sb[:, :], 1.0)
                nc.vector.reciprocal(r1[:, :], t1[:, :])
                nc.scalar.activation(lnv[:, :], n2sb[:, :], AF.Ln)
                nc.scalar.activation(rt[:, :], lnv[:, :], AF.Exp, scale=0.5)
                nc.vector.tensor_scalar_add(t2[:, :], rt[:, :], EPS)
                nc.vector.reciprocal(r2[:, :], t2[:, :])
                nc.vector.tensor_tensor(alpha_bc[:, :], n2sb[:, :], r1[:, :],
                                        MULT)
                nc.vector.tensor_tensor(alpha_bc[:, :], alpha_bc[:, :],
                                        r2[:, :], MULT)

                # ---- v = alpha * s ----
                nc.vector.tensor_scalar(sv[:, :], s_sb[:, :], alpha_bc[:, :],
                                        None, MULT)

                if r < NROUT - 1 and STAGE >= 3:
                    nc.vector.tensor_copy(sv_bf[:, :], sv[:, :])
                    for bt in range(2):
                        # g = sum_i u_hat * v   -> logits update
                        svb = sv_bf[:, bt * 160:(bt + 1) * 160].rearrange(
                            "p (i d) -> p i d", i=16, d=10)
                        nc.vector.tensor_tensor(
                            y_ap(),
                            uh_ap(bt),
                            svb[:, None, None, :, :].to_broadcast(
                                (128, G, 3, 16, 10)),
                            MULT,
                        )
                        yi = Y[:, :].rearrange(
                            "p (pp i d) -> p pp i d", pp=144, i=16, d=10)
                        for sz in (8, 4, 2):
                            nc.vector.tensor_tensor(
                                yi[:, :, 0:sz, :], yi[:, :, 0:sz, :],
                                yi[:, :, sz:2 * sz, :], ADD)
                        nc.vector.tensor_tensor(
                            gf32[:, :].rearrange("p (pp d) -> p pp d",
                                                 pp=144, d=10),
                            yi[:, :, 0, :], yi[:, :, 1, :], ADD)
                        # L += g  (g from sv already carries the alpha scale)
                        Ls = Lb[:, bt * 1440:(bt + 1) * 1440]
                        nc.vector.tensor_tensor(Ls, Ls, gf32[:, :], ADD)
                        nc.scalar.activation(expL[:, :], Ls, AF.Exp)
                        nc.vector.tensor_reduce(
                            zbuf[:, :],
                            expL[:, :].rearrange("p (pp d) -> p pp d",
                                                 pp=144, d=10),
                            mybir.AxisListType.X, ADD)
                        nc.vector.reciprocal(zrec[:, :], zbuf[:, :])
                        nc.vector.tensor_tensor(
                            cpd[:, bt * 1440:(bt + 1) * 1440].rearrange(
                                "p (pp d) -> p pp d", pp=144, d=10),
                            expL[:, :].rearrange("p (pp d) -> p pp d",
                                                 pp=144, d=10),
                            zrec[:, :, None].to_broadcast((128, 144, 10)),
                            MULT,
                        )

            # ---- write v out:  v[b,d,i] = sv[b, i*10+d] ----
            vout_sb = pp.tile([128, 160], F32)
            vsrc = sv if STAGE >= 2 else s_sb
            for bt in range(2):
                nc.vector.tensor_copy(
                    vout_sb[:, :].rearrange("p (d i) -> p d i", d=10, i=16),
                    vsrc[:, bt * 160:(bt + 1) * 160].rearrange(
                        "p (i d) -> p d i", i=16, d=10))
                nc.sync.dma_start(v_d.ap()[bt * 128:(bt + 1) * 128, :, :],
                                  vout_sb[:, :])

    nc.compile()
    return nc


def prep_inputs(primary_caps: np.ndarray, W: np.ndarray):
    """Host-side shard + layout prep. Returns in_maps for the 8 cores."""
    x = np.asarray(primary_caps, dtype=np.float32)
    Wf = np.asarray(W, dtype=np.float32)
    in_maps = []
    for k in range(CORES):
        pk = slice(k * PL, (k + 1) * PL)
        xk = x[:, pk, :]                       # [256, 144, 8]
        Wk = Wf[:, pk, :, :]                   # [10, 144, 16, 8]

        # x_l  [128, NQ*256]: row (gg*32 + ph*8 + j), col (q*256 + b)
        # group g = q*4+gg owns p_local = 3g..3g+2; ph==3 rows are padding
        xg = xk.reshape(B, G, 3, J)            # b, g, ph, j
        xl = np.zeros((G, 4, J, B), dtype=np.float32)
        xl[:, :3] = xg.transpose(1, 2, 3, 0)
        xl = xl.reshape(NQ, 4, 32, B).transpose(1, 2, 0, 3)  # gg, 32, q, b
        xl = xl.reshape(128, NQ * B)

        # w_bd [128, NQ*480]: row (gg*32 + ph*8 + j),
        #                     col (q*480 + php*160 + i*10 + d), delta(ph,php)
        Wt = Wk.reshape(D, G, 3, I, J).transpose(1, 2, 4, 3, 0)  # g,ph,j,i,d
        wbd = np.zeros((G, 4, J, 3, I, D), dtype=np.float32)
        for ph in range(3):
            wbd[:, ph, :, ph, :, :] = Wt[:, ph]
        wbd = wbd.reshape(NQ, 4, 32, GW).transpose(1, 2, 0, 3)
        wbd = wbd.reshape(128, NQ * GW)

        # x_pj [128, 9*256]: row = (p*8+j) % 128, col (kt*256 + b)
        xpj = xk.transpose(1, 2, 0).reshape(PL * J, B)
        xpj = xpj.reshape(NKT, 128, B).transpose(1, 0, 2).reshape(128, NKT * B)

        # w_fl [128, 9*160]: row = (p*8+j) % 128, col (kt*160 + i*10 + d)
        wfl = Wk.transpose(1, 3, 2, 0).reshape(PL * J, I * D)
        wfl = wfl.reshape(NKT, 128, I * D).transpose(1, 0, 2)
        wfl = wfl.reshape(128, NKT * I * D)

        bf = ml_dtypes.bfloat16
        in_maps.append({
            "x_l": xl.astype(bf),
            "w_bd": wbd.astype(bf),
            "x_pj": xpj.astype(bf),
            "w_fl": wfl.astype(bf),
        })
    return in_maps


_NC_CACHE = None


def get_program():
    global _NC_CACHE
    if _NC_CACHE is None:
        _NC_CACHE = build_program()
    return _NC_CACHE


def kernel(primary_caps: np.ndarray, W: np.ndarray) -> np.ndarray:
    nc = get_program()
    in_maps = prep_inputs(primary_caps, W)
    res = run_bass_kernel_spmd(nc, in_maps, core_ids=list(range(CORES)))
    return np.asarray(res.results[0]["v_out"], dtype=np.float32)


if __name__ == "__main__":
    rng = np.random.default_rng(0)
    x = rng.standard_normal((B, P, J), dtype=np.float32)
    W = rng.standard_normal((D, P, I, J), dtype=np.float32).astype(np.float32)
    out = kernel(x, W)
    print("out", out.shape, out.dtype, float(np.abs(out).mean()))



# revision 18
# speedup vs baseline: 1.0566x; 1.0566x over previous
"""CapsNet DigitCaps routing kernel for 8 TRN2 NeuronCores.

Strategy: shard the 1152 primary capsules across the 8 cores (144 each),
keep the full batch (256) on every core. Each core builds its slice of
u_hat = einsum('dpij,bpj->bdpi') once on the TensorEngine (block-diagonal
weight trick: 3 primary capsules per matmul, K=32 with zero padding,
4-way tile_position row-tiling, N=480 = one PSUM bank) and keeps it
resident in SBUF as bf16 (11.8MB).

Routing iterations (vs the earlier all-DVE version):
  s-phase (r=1,2):  Y = c*u_hat broadcast-multiply + pairwise add tree,
    split ~78/22 between DVE and the Pool engine (Pool is 4x slower per
    element but runs in parallel).
  g-phase (r=0,1):  computed in j-space instead of i-space:
       g[b,d,p] = sum_j x[b,p,j] * wv[b,d,p,j],
       wv[b,d,p,j] = sum_i W[d,p,i,j] * s[b,d,i]
    wv is a TensorEngine matmul (k=i=16, lhsT = s^T via PE transpose,
    rhs = per-digit W slices [16, j*p]); the alpha squash scale is folded
    into the PSUM->SBUF drain (scale-multiply). DVE then only does a
    half-size multiply (J=8 vs I=16) plus a 3-level j-tree.
  Build PSUM drains are spread across DVE/ACT/Pool.

The reference's _squash uses a GLOBAL Frobenius norm over the whole
[B,D,1,16] s tensor, which couples all batch elements and hence all
shards: each iteration does one small (164KB) AllReduce of the per-core
partial s, after which every core computes the identical squash scale
alpha = n2 / ((n2+1) * (sqrt(n2)+eps)) and proceeds identically.

Layouts (per core, SBUF partition dim first):
  u_hat  [128, 2*48*480] bf16   row=b%128, col = bt*23040 + g*480 + ph*160 + i*10 + d
                                 (bt=b//128, p_local = 3*g+ph)
  L      [128, 2*1440] f32      col = bt*1440 + d*144 + p   (d-major!)
  cpd    [128, 2*1440] bf16     col = bt*1440 + p*10 + d    (p-major, d innermost)
  wv     [128, 11520]  bf16     col = d*1152 + j*144 + p    (per-bt scratch)
  s,sv   [128, 2*160]  f32      col = bt*160 + i*10 + d
"""

import os
import sys

for _p in ("/opt/trn_rl_repo",):
    if _p not in sys.path and os.path.isdir(_p):
        sys.path.insert(0, _p)

import numpy as np
import ml_dtypes

import concourse.bass as bass
import concourse.bacc as bacc
import concourse.mybir as mybir
import concourse.tile as tile
from concourse.bass_utils import run_bass_kernel_spmd

F32 = mybir.dt.float32
BF16 = mybir.dt.bfloat16
MULT = mybir.AluOpType.mult
ADD = mybir.AluOpType.add
AF = mybir.ActivationFunctionType

B, D, P, I, J = 256, 10, 1152, 16, 8
CORES = 8
PL = P // CORES          # 144 local primary capsules
G = PL // 3              # 48 groups of 3 (block-diag build, N=480 = one PSUM bank)
GW = 3 * I * D           # 480 cols per group
NQ = G // 4              # 12 q-tiles of 4 groups stacked on 128 partitions
NKT = (PL * J) // 128    # 9 k-tiles of 128 (p,j)-rows
EPS = 1e-7
NROUT = 3

# Pool-engine share of the big elementwise ops (in 16-col blocks of 144).
POOL_BLKS = int(os.environ.get("BASSCAPS_POOL_BLKS", "28"))
# Pool-engine share of the g-phase (in digits of 10).
POOL_D = int(os.environ.get("BASSCAPS_POOL_D", "2"))
# Debug bisection: 0=build+s0, 1=+AllReduce, 2=+alpha/sv, 3=+g/L/softmax, 4=full
STAGE = int(os.environ.get("BASSCAPS_STAGE", "4"))
# Replace the AllReduce with a local DRAM copy (for TimelineSim profiling).
NO_CC = os.environ.get("BASSCAPS_NO_CC", "0") == "1"


def _pair_reduce(nc_engine, view_fn, nblk, blk_cols, add_op):
    """Pairwise-sum blocks [0, nblk) into block 0 via repeated halving.

    view_fn(lo, n) must return an AP covering blocks [lo, lo+n).
    Handles odd sizes by folding the tail block in at the next level.
    """
    carry = []
    n = nblk
    while n > 1:
        h = n // 2
        nc_engine.tensor_tensor(view_fn(0, h), view_fn(0, h), view_fn(h, h), add_op)
        if n % 2 == 1:
            carry.append(2 * h)
        n = h
    for cblk in carry:
        nc_engine.tensor_tensor(view_fn(0, 1), view_fn(0, 1), view_fn(cblk, 1), add_op)


def build_program():
    nc = bacc.Bacc("TRN2", target_bir_lowering=False, debug=False,
                   num_devices=CORES)

    # Per-core DRAM inputs (host pre-arranged, bf16).
    xl_d = nc.dram_tensor("x_l", [128, NQ * 256], BF16, kind="ExternalInput")
    wbd_d = nc.dram_tensor("w_bd", [128, NQ * GW], BF16, kind="ExternalInput")
    xpj_d = nc.dram_tensor("x_pj", [128, NKT * 256], BF16, kind="ExternalInput")
    wfl_d = nc.dram_tensor("w_fl", [128, NKT * 160], BF16, kind="ExternalInput")
    wg_d = nc.dram_tensor("w_g", [16, D * J * PL], BF16, kind="ExternalInput")
    xbp_d = nc.dram_tensor("x_bp", [128, 2 * J * PL], BF16, kind="ExternalInput")
    ident_d = nc.dram_tensor("ident", [128, 128], BF16, kind="ExternalInput")
    v_d = nc.dram_tensor("v_out", [B, D, I], F32, kind="ExternalOutput")

    DVE_BLKS = PL - POOL_BLKS           # 116 blocks of 16 cols for DVE
    DVE_D = D - POOL_D                  # 8 digits for DVE in g-phase

    with tile.TileContext(nc) as tc:
        with (
            tc.tile_pool(name="persist", bufs=1) as pp,
            tc.tile_pool(name="build", bufs=3) as bp,
            tc.tile_pool(name="psB", bufs=1, space=bass.MemorySpace.PSUM) as psB,
            tc.tile_pool(name="dram", bufs=1, space=bass.MemorySpace.DRAM) as dp,
        ):
            uh = pp.tile([128, 2 * G * GW], BF16)
            Lb = pp.tile([128, 2 * 1440], F32)
            cpd = pp.tile([128, 2 * 1440], BF16)
            expL = pp.tile([128, 1440], BF16)
            s_sb = pp.tile([128, 2 * 160], F32)
            # s^T staging: col = bt*320 + d*32 + i (16 pad cols per digit so
            # each digit's transposed rows start 32-aligned in PSUM)
            s_bf = pp.tile([128, 2 * 320], BF16)
            sv = pp.tile([128, 2 * 160], F32)
            Y = pp.tile([128, D * J * PL], BF16)   # s/g-phase scratch (11520)
            wv = Y           # g-phase scratch aliases Y (disjoint in time)
            wg = pp.tile([16, D * J * PL], BF16)
            xbp = pp.tile([128, 2 * J * PL], BF16)
            ident = pp.tile([128, 128], BF16)
            # per-digit s^T tiles [16(i), 2*128(bt,b)] — each at base
            # partition 0 (PE lhsT requires base partition 0/32/64)
            vtd = [pp.tile([16, 2 * 128], BF16, name=f"vt{d}")
                   for d in range(D)]
            zs = pp.tile([128, 5 * 144], BF16)
            zrec = pp.tile([128, 144], F32)
            acc = pp.tile([128, 1], F32)
            ones = pp.tile([128, 128], F32)
            ttr_junk = pp.tile([128, 320], F32)
            n2sb = pp.tile([128, 1], F32)
            t1 = pp.tile([128, 1], F32)
            r1 = pp.tile([128, 1], F32)
            lnv = pp.tile([128, 1], F32)
            rt = pp.tile([128, 1], F32)
            t2 = pp.tile([128, 1], F32)
            r2 = pp.tile([128, 1], F32)
            alpha_bc = pp.tile([128, 1], F32)

            bounce_in = dp.tile([B, 160], F32)
            bounce_out = dp.tile([B, 160], F32)

            nc.vector.memset(ones[:, :], 1.0)
            nc.gpsimd.memset(Lb[:, :], 0.0)
            nc.vector.memset(s_bf[:, :], 0.0)

            # ---- load the small operands ----
            xpj = bp.tile([128, NKT * 256], BF16, tag="xpj")
            wfl = bp.tile([128, NKT * 160], BF16, tag="wfl")
            nc.sync.dma_start(xpj[:, :], xpj_d.ap())
            nc.sync.dma_start(wfl[:, :], wfl_d.ap())
            nc.sync.dma_start(wg[:, :], wg_d.ap())
            nc.sync.dma_start(xbp[:, :], xbp_d.ap())
            nc.sync.dma_start(ident[:, :], ident_d.ap())

            # ---- s0 = 0.1 * sum_p u_hat  (direct from x, W) ----
            for bt in range(2):
                ps0 = psB.tile([128, 160], F32, tag="s0")
                for kt in range(NKT):
                    nc.tensor.matmul(
                        ps0[:, :],
                        xpj[:, kt * 256 + bt * 128: kt * 256 + bt * 128 + 128],
                        wfl[:, kt * 160:(kt + 1) * 160],
                        start=(kt == 0), stop=(kt == NKT - 1),
                    )
                nc.scalar.mul(s_sb[:, bt * 160:(bt + 1) * 160], ps0[:, :], 0.1)

            # ---- build u_hat: 4-way row-tiled block-diag matmuls ----
            # group g = 3 primary capsules; lhsT = x block [32, 128]
            # (rows = (ph, j), ph==3 padding), rhs = W block-diag [32, 480].
            # Drains cycle DVE/ACT/Pool (3/3/2 per q-tile).
            with tc.tile_pool(name="psA", bufs=4, space=bass.MemorySpace.PSUM) as psA:
                drain_cycle = [0, 1, 0, 1, 0, 1, 0, 1]
                for q in range(NQ):
                    xlq = bp.tile([128, 256], BF16, tag="xl")
                    wbq = bp.tile([128, GW], BF16, tag="wb")
                    nc.sync.dma_start(xlq[:, :], xl_d.ap()[:, q * 256:(q + 1) * 256])
                    nc.sync.dma_start(wbq[:, :], wbd_d.ap()[:, q * GW:(q + 1) * GW])
                    for bt in range(2):
                        for gg in range(4):
                            g = q * 4 + gg
                            ps = psA.tile([128, GW], F32, tag="bld")
                            nc.tensor.matmul(
                                ps[:, :],
                                xlq[32 * gg:32 * (gg + 1),
                                    bt * 128:(bt + 1) * 128],
                                wbq[32 * gg:32 * (gg + 1), :],
                                start=True, stop=True,
                                tile_position=(32 * gg, 0),
                            )
                            dst = uh[:, bt * (G * GW) + g * GW:
                                     bt * (G * GW) + (g + 1) * GW]
                            eng = drain_cycle[(bt * 4 + gg) % 8]
                            if eng == 0:
                                nc.vector.tensor_copy(dst, ps[:, :])
                            elif eng == 1:
                                nc.scalar.copy(dst, ps[:, :])
                            else:
                                nc.gpsimd.tensor_copy(dst, ps[:, :])

            # wv PSUM pools open after the build pool closes (bank budget).
            with (
                tc.tile_pool(name="psW", bufs=2,
                             space=bass.MemorySpace.PSUM) as psW,
                tc.tile_pool(name="psT", bufs=1,
                             space=bass.MemorySpace.PSUM) as psT,
            ):

                def uh_ap(bt, lo, n):
                    """u_hat view: blocks of 16 cols -> (blk, i, d)."""
                    return uh[:, bt * (G * GW) + lo * 160:
                              bt * (G * GW) + (lo + n) * 160].rearrange(
                        "p (k i d) -> p k i d", k=n, i=16, d=10)

                def y_ap(lo, n):
                    return Y[:, lo * 160:(lo + n) * 160].rearrange(
                        "p (k i d) -> p k i d", k=n, i=16, d=10)

                for r in range(NROUT if STAGE >= 4 else 1):
                    if r > 0:
                        # ---- s_partial = sum_p c * u_hat ----
                        # two 72-block chunks per bt (Y holds 72 blocks)
                        CH = 72
                        DVE_C = CH - POOL_BLKS // 2      # 58
                        POOL_C = POOL_BLKS // 2          # 14
                        for bt in range(2):
                            cb = cpd[:, bt * 1440:(bt + 1) * 1440].rearrange(
                                "p (pp d) -> p pp d", pp=144, d=10)

                            def cview(lo, n):
                                return cb[:, lo:lo + n, None, :].to_broadcast(
                                    (128, n, 16, 10))

                            ss = s_sb[:, bt * 160:(bt + 1) * 160]
                            for ci in range(2):
                                clo = ci * CH
                                nc.vector.tensor_tensor(
                                    y_ap(0, DVE_C), uh_ap(bt, clo, DVE_C),
                                    cview(clo, DVE_C), MULT)
                                nc.gpsimd.tensor_tensor(
                                    y_ap(DVE_C, POOL_C),
                                    uh_ap(bt, clo + DVE_C, POOL_C),
                                    cview(clo + DVE_C, POOL_C), MULT)

                                yb = Y[:, :CH * 160].rearrange(
                                    "p (k x) -> p k x", k=CH, x=160)

                                def yview_d(lo, n):
                                    return yb[:, lo:lo + n, :]

                                def yview_p(lo, n):
                                    return yb[:, DVE_C + lo:DVE_C + lo + n, :]

                                _pair_reduce(nc.vector, yview_d, DVE_C, 160, ADD)
                                _pair_reduce(nc.gpsimd, yview_p, POOL_C, 160, ADD)
                                nc.vector.tensor_tensor(
                                    yb[:, 0, :], yb[:, 0, :], yb[:, DVE_C, :],
                                    ADD)
                                if ci == 0:
                                    nc.vector.tensor_copy(ss, yb[:, 0, :])
                                else:
                                    nc.vector.tensor_tensor(
                                        ss, ss, yb[:, 0, :], ADD)

                    # ---- AllReduce partial s over the 8 cores ----
                    if STAGE < 1:
                        break
                    nc.sync.dma_start(
                        bounce_in[:, :].rearrange("(t b) x -> b t x", t=2, b=128),
                        s_sb[:, :].rearrange("b (t x) -> b t x", t=2, x=160))
                    if NO_CC:
                        nc.sync.dma_start(bounce_out[:, :], bounce_in[:, :])
                    else:
                        nc.gpsimd.collective_compute(
                            "AllReduce", ADD,
                            ins=[bounce_in.opt()],
                            outs=[bounce_out.opt()],
                            replica_groups=[list(range(CORES))],
                        )
                    nc.sync.dma_start(
                        s_sb[:, :].rearrange("b (t x) -> b t x", t=2, x=160),
                        bounce_out[:, :].rearrange("(t b) x -> b t x", t=2, b=128))

                    if STAGE < 2:
                        break
                    # ---- alpha = n2 / ((n2+1)(sqrt(n2)+eps)) , n2 = sum s^2 ----
                    nc.scalar.activation(ttr_junk[:, :], s_sb[:, :], AF.Square,
                                         accum_out=acc[:, :])
                    psn = psB.tile([128, 1], F32, tag="n2")
                    nc.tensor.matmul(psn[:, :], ones[:, :], acc[:, :],
                                     start=True, stop=True)
                    nc.vector.tensor_copy(n2sb[:, :], psn[:, :])
                    nc.vector.tensor_scalar_add(t1[:, :], n2sb[:, :], 1.0)
                    nc.vector.reciprocal(r1[:, :], t1[:, :])
                    nc.scalar.activation(lnv[:, :], n2sb[:, :], AF.Ln)
                    nc.scalar.activation(rt[:, :], lnv[:, :], AF.Exp, scale=0.5)
                    nc.vector.tensor_scalar_add(t2[:, :], rt[:, :], EPS)
                    nc.vector.reciprocal(r2[:, :], t2[:, :])
                    nc.vector.tensor_tensor(alpha_bc[:, :], n2sb[:, :], r1[:, :],
                                            MULT)
                    nc.vector.tensor_tensor(alpha_bc[:, :], alpha_bc[:, :],
                                            r2[:, :], MULT)

                    # ---- v = alpha * s (final output path) ----
                    nc.vector.tensor_scalar(sv[:, :], s_sb[:, :], alpha_bc[:, :],
                                            None, MULT)

                    if r < NROUT - 1 and STAGE >= 3:
                        # ==== g-phase in j-space ====
                        # s^T via PE transpose: rows (d,i) d-major, cols b.
                        nc.vector.tensor_copy(
                            s_bf[:, :].rearrange("p (t d i) -> p t d i",
                                                 t=2, d=10, i=32)[:, :, :, :16],
                            s_sb[:, :].rearrange("p (t i d) -> p t d i",
                                                 t=2, i=16, d=10))
                        for bt in range(2):
                            for rnd, (dlo, nd) in enumerate(
                                    ((0, 4), (4, 4), (8, 2))):
                                ptv = psT.tile([128, 128], BF16, tag="vt")
                                nc.tensor.transpose(
                                    ptv[:32 * nd, :],
                                    s_bf[:, bt * 320 + dlo * 32:
                                         bt * 320 + (dlo + nd) * 32],
                                    ident[:, :])
                                for dd in range(nd):
                                    nc.vector.tensor_copy(
                                        vtd[dlo + dd][:, bt * 128:
                                                      (bt + 1) * 128],
                                        ptv[32 * dd:32 * dd + 16, :])

                        for bt in range(2):
                            # wv[d] = s_d^T @ W_g[d]  (k=16), alpha in drain
                            wv_drain = [1, 1, 1, 0, 1, 1, 1, 0, 1, 1]
                            JP2 = J * PL // 3
                            for d in range(D):
                                for h in range(3):
                                    psv = psW.tile([128, JP2], F32, tag="wv")
                                    nc.tensor.matmul(
                                        psv[:, :],
                                        vtd[d][:, bt * 128:(bt + 1) * 128],
                                        wg[:, d * (J * PL) + h * JP2:
                                           d * (J * PL) + (h + 1) * JP2],
                                        start=True, stop=True,
                                    )
                                    dstv = wv[:, d * (J * PL) + h * JP2:
                                              d * (J * PL) + (h + 1) * JP2]
                                    eng = wv_drain[d]
                                    if eng == 0:
                                        nc.vector.tensor_scalar(
                                            dstv, psv[:, :], alpha_bc[:, :1],
                                            None, MULT)
                                    elif eng == 1:
                                        nc.scalar.mul(dstv, psv[:, :],
                                                      alpha_bc[:, :1])
                                    else:
                                        nc.gpsimd.tensor_scalar(
                                            dstv, psv[:, :], alpha_bc[:, :1],
                                            None, MULT)

                            # g = sum_j x * wv : multiply + j-tree, split by d
                            wv4 = wv[:, :D * J * PL].rearrange(
                                "p (d j q) -> p d j q", d=D, j=J, q=PL)
                            xb = xbp[:, bt * (J * PL):(bt + 1) * (J * PL)]
                            xv = xb.rearrange("p (j q) -> p j q", j=J, q=PL)

                            def xview(dlo, dn):
                                return xv[:, None, :, :].to_broadcast(
                                    (128, dn, J, PL))

                            nc.vector.tensor_tensor(
                                wv4[:, 0:DVE_D], wv4[:, 0:DVE_D],
                                xview(0, DVE_D), MULT)
                            nc.gpsimd.tensor_tensor(
                                wv4[:, DVE_D:D], wv4[:, DVE_D:D],
                                xview(DVE_D, POOL_D), MULT)
                            for eng, dlo, dn in ((nc.vector, 0, DVE_D),
                                                 (nc.gpsimd, DVE_D, POOL_D)):
                                for sz in (4, 2, 1):
                                    eng.tensor_tensor(
                                        wv4[:, dlo:dlo + dn, 0:sz, :],
                                        wv4[:, dlo:dlo + dn, 0:sz, :],
                                        wv4[:, dlo:dlo + dn, sz:2 * sz, :], ADD)

                            # L += g   (L is d-major [d, p], f32)
                            Ls = Lb[:, bt * 1440:(bt + 1) * 1440].rearrange(
                                "p (d q) -> p d q", d=D, q=PL)
                            nc.vector.tensor_tensor(
                                Ls[:, 0:DVE_D], Ls[:, 0:DVE_D],
                                wv4[:, 0:DVE_D, 0, :], ADD)
                            nc.gpsimd.tensor_tensor(
                                Ls[:, DVE_D:D], Ls[:, DVE_D:D],
                                wv4[:, DVE_D:D, 0, :], ADD)

                            # softmax over d: exp writes (p,d)-transposed
                            nc.scalar.activation(
                                expL[:, :].rearrange("p (q d) -> p d q",
                                                     q=PL, d=D),
                                Lb[:, bt * 1440:(bt + 1) * 1440].rearrange(
                                    "p (d q) -> p d q", d=D, q=PL),
                                AF.Exp)
                            ev = expL[:, :].rearrange("p (q d) -> p q d",
                                                      q=PL, d=D)
                            # z-tree over d into zs scratch (keeps expL intact)
                            zv = zs[:, :].rearrange("p (q d) -> p q d",
                                                    q=PL, d=5)
                            nc.vector.tensor_tensor(
                                zv[:, :, :], ev[:, :, 0:5], ev[:, :, 5:10],
                                ADD)
                            nc.vector.tensor_tensor(
                                zv[:, :, 0:2], zv[:, :, 0:2], zv[:, :, 2:4],
                                ADD)
                            nc.vector.tensor_tensor(
                                zv[:, :, 0:1], zv[:, :, 0:1], zv[:, :, 1:2],
                                ADD)
                            nc.vector.tensor_tensor(
                                zv[:, :, 0:1], zv[:, :, 0:1], zv[:, :, 4:5],
                                ADD)
                            nc.vector.reciprocal(zrec[:, :, None], zv[:, :, 0:1])
                            nc.vector.tensor_tensor(
                                cpd[:, bt * 1440:(bt + 1) * 1440].rearrange(
                                    "p (q d) -> p q d", q=PL, d=D),
                                expL[:, :].rearrange("p (q d) -> p q d",
                                                     q=PL, d=D),
                                zrec[:, :, None].to_broadcast((128, PL, D)),
                                MULT)

                # ---- write v out:  v[b,d,i] = sv[b, i*10+d] ----
                vout_sb = pp.tile([128, 160], F32)
                vsrc = sv if STAGE >= 2 else s_sb
                for bt in range(2):
                    nc.vector.tensor_copy(
                        vout_sb[:, :].rearrange("p (d i) -> p d i", d=10, i=16),
                        vsrc[:, bt * 160:(bt + 1) * 160].rearrange(
                            "p (i d) -> p d i", i=16, d=10))
                    nc.sync.dma_start(v_d.ap()[bt * 128:(bt + 1) * 128, :, :],
                                      vout_sb[:, :])

    nc.compile()
    return nc


def prep_inputs(primary_caps: np.ndarray, W: np.ndarray):
    """Host-side shard + layout prep. Returns in_maps for the 8 cores."""
    x = np.asarray(primary_caps, dtype=np.float32)
    Wf = np.asarray(W, dtype=np.float32)
    bf = ml_dtypes.bfloat16
    ident = np.eye(128, dtype=np.float32).astype(bf)
    in_maps = []
    for k in range(CORES):
        pk = slice(k * PL, (k + 1) * PL)
        xk = x[:, pk, :]                       # [256, 144, 8]
        Wk = Wf[:, pk, :, :]                   # [10, 144, 16, 8]

        # x_l  [128, NQ*256]: row (gg*32 + ph*8 + j), col (q*256 + b)
        # group g = q*4+gg owns p_local = 3g..3g+2; ph==3 rows are padding
        xg = xk.reshape(B, G, 3, J)            # b, g, ph, j
        xl = np.zeros((G, 4, J, B), dtype=np.float32)
        xl[:, :3] = xg.transpose(1, 2, 3, 0)
        xl = xl.reshape(NQ, 4, 32, B).transpose(1, 2, 0, 3)  # gg, 32, q, b
        xl = xl.reshape(128, NQ * B)

        # w_bd [128, NQ*480]: row (gg*32 + ph*8 + j),
        #                     col (q*480 + php*160 + i*10 + d), delta(ph,php)
        Wt = Wk.reshape(D, G, 3, I, J).transpose(1, 2, 4, 3, 0)  # g,ph,j,i,d
        wbd = np.zeros((G, 4, J, 3, I, D), dtype=np.float32)
        for ph in range(3):
            wbd[:, ph, :, ph, :, :] = Wt[:, ph]
        wbd = wbd.reshape(NQ, 4, 32, GW).transpose(1, 2, 0, 3)
        wbd = wbd.reshape(128, NQ * GW)

        # x_pj [128, 9*256]: row = (p*8+j) % 128, col (kt*256 + b)
        xpj = xk.transpose(1, 2, 0).reshape(PL * J, B)
        xpj = xpj.reshape(NKT, 128, B).transpose(1, 0, 2).reshape(128, NKT * B)

        # w_fl [128, 9*160]: row = (p*8+j) % 128, col (kt*160 + i*10 + d)
        wfl = Wk.transpose(1, 3, 2, 0).reshape(PL * J, I * D)
        wfl = wfl.reshape(NKT, 128, I * D).transpose(1, 0, 2)
        wfl = wfl.reshape(128, NKT * I * D)

        # w_g [16, D*J*PL]: row i, col (d*1152 + j*144 + p) = W[d,p,i,j]
        wg = Wk.transpose(2, 0, 3, 1).reshape(I, D * J * PL)

        # x_bp [128, 2*J*PL]: row b%128, col (bt*1152 + j*144 + p)
        xbp = xk.transpose(0, 2, 1).reshape(B, J * PL)       # b, (j, p)
        xbp = xbp.reshape(2, 128, J * PL).transpose(1, 0, 2).reshape(
            128, 2 * J * PL)

        in_maps.append({
            "x_l": xl.astype(bf),
            "w_bd": wbd.astype(bf),
            "x_pj": xpj.astype(bf),
            "w_fl": wfl.astype(bf),
            "w_g": wg.astype(bf),
            "x_bp": xbp.astype(bf),
            "ident": ident,
        })
    return in_maps


_NC_CACHE = None


def get_program():
    global _NC_CACHE
    if _NC_CACHE is None:
        _NC_CACHE = build_program()
    return _NC_CACHE


def kernel(primary_caps: np.ndarray, W: np.ndarray) -> np.ndarray:
    nc = get_program()
    in_maps = prep_inputs(primary_caps, W)
    res = run_bass_kernel_spmd(nc, in_maps, core_ids=list(range(CORES)))
    return np.asarray(res.results[0]["v_out"], dtype=np.float32)


if __name__ == "__main__":
    rng = np.random.default_rng(0)
    x = rng.standard_normal((B, P, J), dtype=np.float32)
    W = rng.standard_normal((D, P, I, J), dtype=np.float32).astype(np.float32)
    out = kernel(x, W)
    print("out", out.shape, out.dtype, float(np.abs(out).mean()))
